# revision 1
# baseline (speedup 1.0000x reference)
"""Bass/Tile kernel for nn_AligningModel on 8 Trainium2 NeuronCores.

Data parallel: 32 samples sharded 4-per-core across 8 cores; all params
replicated.  Inside each core the model runs feature-major ([C, T] layout)
with bf16 matmul operands and fp32 PSUM accumulation:

  mel GLU encoder (4 layers)  -> mel_enc   [384, T]
  phoneme GLU encoder (4)     -> ph_enc    [384, 257]
  -L2^2 attention softmax     -> context   [384, T]   (the |mel|^2 term
      cancels inside softmax, so scores are 2*mel.ph - |ph|^2 - penalty)
  dec GLU (2 layers, 768ch)   -> dec       [768, T]
  logits twice (token-major for log_softmax, feature-major for mel decoder)
  mel GLU decoder (2 layers, 128ch) -> tanh mel preds

Convolutions (k=3, SAME) are shifted matmuls accumulated in PSUM.
"""

import numpy as np
import ml_dtypes

BF = ml_dtypes.bfloat16

B_FULL = 32
T_MEL = 2000
T_PHON = 256
E = 384
V = 256
MEL_DIMS = 80
DEC_H = 128
N_CORES = 8
NS = B_FULL // N_CORES  # samples per core


def _ttiles(T, w=512):
    out, t0 = [], 0
    while t0 < T:
        out.append((t0, min(w, T - t0)))
        t0 += w
    return out


def build(ns=NS, T=T_MEL, TPH=T_PHON, teffs=None):
    """Build and compile the per-core Bass kernel (ns samples, seq len T)."""
    import concourse.bacc as bacc
    import concourse.tile as tile
    import concourse.mybir as mybir
    from concourse.alu_op_type import AluOpType as aop
    from concourse.masks import make_identity
    from concourse import bass_isa

    f32 = mybir.dt.float32
    bf16 = mybir.dt.bfloat16
    AF = mybir.ActivationFunctionType
    AX = mybir.AxisListType

    if teffs is None:
        teffs = [T] * ns
    assert len(teffs) == ns and all(128 <= t <= T and t % 128 == 0 or t == T
                                    for t in teffs)
    S = TPH + 1          # phoneme positions incl. prepended blank
    TP = T + 2           # zero border column each side
    SP = S + 2
    TT = _ttiles(T)          # 512-wide t tiles
    TBLK = _ttiles(T, 128)   # 128-wide t blocks
    SBLK = _ttiles(S, 128)   # s chunks
    EB = E // 128            # 3
    HB = 2 * E // 128        # 6
    VB = V // 128            # 2
    XMW = 516                # xm chunk slot width (512 + 2 halo + margin)

    nc = bacc.Bacc("TRN2", debug=False, target_bir_lowering=False)

    # ---------------- DRAM I/O ----------------
    d_mels = nc.dram_tensor("mels", [ns, MEL_DIMS, T], bf16, kind="ExternalInput").ap()
    d_embph = nc.dram_tensor("embph", [ns, E, SP], bf16, kind="ExternalInput").ap()
    d_melmask = nc.dram_tensor("melmask", [ns, TP], bf16, kind="ExternalInput").ap()
    d_phmask = nc.dram_tensor("phmask", [ns, SP], bf16, kind="ExternalInput").ap()
    d_phpen = nc.dram_tensor("phpen", [ns, S], f32, kind="ExternalInput").ap()

    d_meproj = nc.dram_tensor("me_proj", [MEL_DIMS, E], bf16, kind="ExternalInput").ap()
    d_mew = nc.dram_tensor("me_w", [4, 3, E, 2 * E], bf16, kind="ExternalInput").ap()
    d_pew = nc.dram_tensor("pe_w", [4, 3, E, 2 * E], bf16, kind="ExternalInput").ap()
    d_pdw = nc.dram_tensor("pd_w", [2, 3, 2 * E, 4 * E], bf16, kind="ExternalInput").ap()
    d_mdw = nc.dram_tensor("md_w", [2, 3, DEC_H, 2 * DEC_H], bf16, kind="ExternalInput").ap()
    d_pdlin = nc.dram_tensor("pd_lin", [2 * E, V], bf16, kind="ExternalInput").ap()
    d_mdproj = nc.dram_tensor("md_proj", [V, DEC_H], bf16, kind="ExternalInput").ap()
    d_mdlin = nc.dram_tensor("md_lin", [DEC_H, MEL_DIMS], bf16, kind="ExternalInput").ap()
    d_pdlinb_row = nc.dram_tensor("pd_lin_b_row", [1, V], bf16, kind="ExternalInput").ap()
    d_mdlinb_row = nc.dram_tensor("md_lin_b_row", [1, MEL_DIMS], bf16, kind="ExternalInput").ap()

    d_meprojb = nc.dram_tensor("me_proj_b", [E], f32, kind="ExternalInput").ap()
    d_meb = nc.dram_tensor("me_b", [4, 2 * E], f32, kind="ExternalInput").ap()
    d_peb = nc.dram_tensor("pe_b", [4, 2 * E], f32, kind="ExternalInput").ap()
    d_pdb = nc.dram_tensor("pd_b", [2, 4 * E], f32, kind="ExternalInput").ap()
    d_mdb = nc.dram_tensor("md_b", [2, 2 * DEC_H], f32, kind="ExternalInput").ap()
    d_mdprojb = nc.dram_tensor("md_proj_b", [DEC_H], f32, kind="ExternalInput").ap()
    d_pdlinb = nc.dram_tensor("pd_lin_b", [V], f32, kind="ExternalInput").ap()

    d_out = nc.dram_tensor("out", [ns, T, V + MEL_DIMS], f32, kind="ExternalOutput").ap()

    with tile.TileContext(nc) as tc:
        cpool = tc.alloc_tile_pool(name="consts", bufs=1)
        wring = tc.alloc_tile_pool(name="wring", bufs=44)
        apool = tc.alloc_tile_pool(name="acts", bufs=1)
        xring = tc.alloc_tile_pool(name="xmring", bufs=12)
        spool = tc.alloc_tile_pool(name="scratch", bufs=2)
        psA = tc.alloc_tile_pool(name="psumA", bufs=6, space="PSUM")
        psB = tc.alloc_tile_pool(name="psumB", bufs=2, space="PSUM")

        def ps_conv(name="ps"):
            # conv accumulators: private ring so sparse phases can't gate them
            return psA.tile([128, 512], f32, tag="psa", name=name)

        def ps_tile(name="ps"):
            return psB.tile([128, 512], f32, tag="psb", name=name)

        def ps_tile_bf(name="psb"):
            # PE transpose writes through in the input dtype
            return psB.tile([128, 512], bf16, tag="psb", name=name)

        # ---------------- constants / resident weights ----------------
        ones_row = cpool.tile([1, 512], bf16, name="ones_row")
        nc.vector.memset(ones_row, 1.0)
        ones_col = cpool.tile([128, 1], bf16, name="ones_col")
        nc.vector.memset(ones_col, 1.0)
        ident = cpool.tile([128, 128], bf16, name="ident")
        make_identity(nc, ident)

        meproj_sb = cpool.tile([MEL_DIMS, E], bf16, name="meproj_sb")
        nc.sync.dma_start(meproj_sb, d_meproj)
        pdlin_sb = cpool.tile([128, HB, V], bf16, name="pdlin_sb")
        for c in range(HB):
            nc.sync.dma_start(pdlin_sb[:, c, :], d_pdlin[c * 128:(c + 1) * 128, :])
        mdproj_sb = cpool.tile([128, VB, DEC_H], bf16, name="mdproj_sb")
        for c in range(VB):
            nc.sync.dma_start(mdproj_sb[:, c, :], d_mdproj[c * 128:(c + 1) * 128, :])
        mdlin_sb = cpool.tile([128, MEL_DIMS], bf16, name="mdlin_sb")
        nc.sync.dma_start(mdlin_sb, d_mdlin)
        pdlinb_row = cpool.tile([1, V], bf16, name="pdlinb_row")
        nc.sync.dma_start(pdlinb_row, d_pdlinb_row)
        mdlinb_row = cpool.tile([1, MEL_DIMS], bf16, name="mdlinb_row")
        nc.sync.dma_start(mdlinb_row, d_mdlinb_row)

        # per-partition bias tables: [128, nlayer, nblocks]
        b_meproj = cpool.tile([128, EB], f32, name="b_meproj")
        nc.sync.dma_start(b_meproj, d_meprojb.rearrange("(a p) -> p a", p=128))
        b_me = cpool.tile([128, 4, HB], f32, name="b_me")
        nc.sync.dma_start(b_me, d_meb.rearrange("l (a p) -> p l a", p=128))
        b_pe = cpool.tile([128, 4, HB], f32, name="b_pe")
        nc.sync.dma_start(b_pe, d_peb.rearrange("l (a p) -> p l a", p=128))
        b_pd = cpool.tile([128, 2, 2 * HB], f32, name="b_pd")
        nc.sync.dma_start(b_pd, d_pdb.rearrange("l (a p) -> p l a", p=128))
        b_md = cpool.tile([128, 2, 2], f32, name="b_md")
        nc.sync.dma_start(b_md, d_mdb.rearrange("l (a p) -> p l a", p=128))
        b_mdproj = cpool.tile([128, 1], f32, name="b_mdproj")
        nc.sync.dma_start(b_mdproj, d_mdprojb.rearrange("(a p) -> p a", p=128))
        b_pdlin = cpool.tile([128, VB], f32, name="b_pdlin")
        nc.sync.dma_start(b_pdlin, d_pdlinb.rearrange("(a p) -> p a", p=128))


        # ---------------- persistent activations (per-sample reuse) ------
        # decx/phx are allocated per sample below (mel half + phx double
        # buffered so sample s+1's encoders overlap sample s's decoder)
        ph2 = [apool.tile([128, 3 * 128], bf16, name=f"ph2_{c}") for c in range(EB)]
        phT = [apool.tile([128, E], bf16, name=f"phT{i}") for i in range(len(SBLK))]
        mdx = [apool.tile([128, TP], bf16, name="mdx0")]
        logitbf = [apool.tile([128, TP], bf16, name=f"logitbf{v}") for v in range(VB)]
        pen_sb = apool.tile([1, 3 * 128], bf16, name="pen_sb")

        def zero_borders(tiles):
            for t_ in tiles:
                w = t_.shape[1]
                nc.vector.memset(t_[:, 0:1], 0.0)
                nc.vector.memset(t_[:, w - 1:w], 0.0)

        zero_borders(mdx)

        # ---------------- helpers ----------------
        def load_conv_weights(dram, l, n_cin, cout_w, name):
            """Stream one conv layer's weights through the ring.

            Returns dict (k, c, j) -> tile of [128, min(768, cout_w)] where j
            indexes 768-wide chunks of the cout dim.
            """
            njc = (cout_w + 767) // 768
            cw = min(768, cout_w)
            tiles = {}
            for k in range(3):
                for c in range(n_cin):
                    for j in range(njc):
                        wt = wring.tile([128, cw], bf16, tag="wconv",
                                        name=f"{name}{l}_{k}_{c}_{j}")
                        nc.sync.dma_start(
                            wt, dram[l, k, c * 128:(c + 1) * 128,
                                     j * cw:(j + 1) * cw])
                        tiles[(k, c, j)] = wt
            return tiles

        def glu_layer(x_tiles, n_cin, tiles_list, TPAD, mask_rep, wa, wg, bias_a, bias_g,
                      pre_chunk_hook=None, pre_tile_hook=None):
            """One masked GLU conv block, in place on x_tiles (bf16).

            wa(k, c, h) / wg(k, c, h) -> lhsT AP [128, 128] for the a/g couts.
            bias_a(h)/bias_g(h) -> [128, 1] fp32 AP.
            Residual uses the masked input (matches reference semantics).
            pre_chunk_hook(i): emitted just before tile i's xm chunks (used to
            interleave the producer of x_tiles' columns, e.g. attention).
            pre_tile_hook(i): emitted before tile i's conv matmuls (used to
            interleave unrelated sparse-PE phases so HAM stays warm).
            """
            n_half = n_cin  # cout == 2*cin for every GLU here
            xm = {}

            def emit_xm(i):
                if pre_chunk_hook is not None:
                    pre_chunk_hook(i)
                t0, W = tiles_list[i]
                cw = min(W + 2, TPAD - t0)
                for c in range(n_cin):
                    xt = xring.tile([128, XMW], bf16, tag="xm", name=f"xm{c}_{i}")
                    nc.vector.tensor_tensor(
                        out=xt[:, :cw], in0=x_tiles[c][:, t0:t0 + cw],
                        in1=mask_rep[:, t0:t0 + cw], op=aop.mult)
                    xm[(c, i)] = xt

            emit_xm(0)
            for i, (t0, W) in enumerate(tiles_list):
                if i + 1 < len(tiles_list):
                    emit_xm(i + 1)
                if pre_tile_hook is not None:
                    pre_tile_hook(i)
                for h in range(n_half):
                    a_ps = ps_conv("a_ps")
                    g_ps = ps_conv("g_ps")
                    nmm = 3 * n_cin
                    for half, ps in ((0, a_ps), (1, g_ps)):
                        idx = 0
                        for k in range(3):
                            for c in range(n_cin):
                                lhsT = wa(k, c, h) if half == 0 else wg(k, c, h)
                                nc.tensor.matmul(
                                    ps[:, :W], lhsT,
                                    xm[(c, i)][:, k:k + W],
                                    start=(idx == 0), stop=(idx == nmm - 1))
                                idx += 1
                    sig = spool.tile([128, 512], f32, tag="sig", bufs=3, name="sig")
                    nc.scalar.activation(sig[:, :W], g_ps[:, :W], AF.Sigmoid,
                                         bias=bias_g(h), scale=1.0)
                    tmp = spool.tile([128, 512], f32, tag="tmp", bufs=3, name="tmp")
                    nc.vector.scalar_tensor_tensor(
                        out=tmp[:, :W], in0=a_ps[:, :W], scalar=bias_a(h),
                        in1=sig[:, :W], op0=aop.add, op1=aop.mult)
                    # x = tmp + xm   (masked residual, exact in bf16)
                    nc.vector.tensor_tensor(
                        out=x_tiles[h][:, 1 + t0:1 + t0 + W], in0=tmp[:, :W],
                        in1=xm[(h, i)][:, 1:1 + W], op=aop.add)

        import concourse.bass as bass

        def bcast_row(row_ap):
            # [1, N] AP -> stride-0 partition broadcast AP [128, N] for DMA
            return bass.AP(tensor=row_ap.tensor, offset=row_ap.offset,
                           ap=[[0, 128]] + [list(d) for d in row_ap.ap[1:]])

        # ================ per-sample pipeline ================
        for s in range(ns):
            # beyond mel_len the masked convs see zeros, so everything is a
            # per-channel constant: compute t < Teff only, broadcast the tail
            Teff = teffs[s]
            TTs = _ttiles(Teff)
            TBLKs = _ttiles(Teff, 128)
            decx = [apool.tile([128, TP], bf16, tag=f"decx{c}",
                               bufs=2 if c < EB else 1, name=f"decx{c}")
                    for c in range(HB)]
            phx = [apool.tile([128, SP], bf16, tag=f"phx{c}", bufs=2,
                              name=f"phx{c}") for c in range(EB)]
            zero_borders(decx + phx)
            if s < 2 and Teff < T:
                for t_ in decx + mdx + logitbf:
                    nc.vector.memset(t_[:, 1 + Teff:TP - 1], 0.0)
            # ---- masks (partition-broadcast via stride-0 DMA) ----
            melmask = spool.tile([128, TP], bf16, tag="melmask", name="melmask")
            nc.sync.dma_start(out=melmask, in_=bcast_row(d_melmask[s:s + 1, :]))
            phmask = spool.tile([128, SP], bf16, tag="phmask", name="phmask")
            nc.sync.dma_start(out=phmask, in_=bcast_row(d_phmask[s:s + 1, :]))

            # ---- phoneme embedding + encoder (first: cheap DMA, fills the
            # pipe while mel data loads) ----
            for c in range(EB):
                nc.sync.dma_start(phx[c], d_embph[s, c * 128:(c + 1) * 128, :])
            for l in range(4):
                wt = load_conv_weights(d_pew, l, EB, 2 * E, "pew")
                glu_layer(phx, EB, [(0, S)], SP, phmask,
                          wa=lambda k, c, h, wt=wt: wt[(k, c, 0)][:, h * 128:(h + 1) * 128],
                          wg=lambda k, c, h, wt=wt: wt[(k, c, 0)][:, (EB + h) * 128:(EB + h + 1) * 128],
                          bias_a=lambda h, l=l: b_pe[:, l, h:h + 1],
                          bias_g=lambda h, l=l: b_pe[:, l, EB + h:EB + h + 1])

            # ---- attention prep: ph2 = 2*ph_enc, p2 = sum(ph^2), phT ----
            p2_ps = ps_tile("p2_ps")
            for c in range(EB):
                nc.vector.tensor_scalar_mul(ph2[c][:, :S], phx[c][:, 1:1 + S], 2.0)
                nc.vector.memset(ph2[c][:, S:], 0.0)
                sq = spool.tile([128, S], bf16, tag="sq", name="sq")
                nc.vector.tensor_tensor(sq[:, :S], phx[c][:, 1:1 + S],
                                        phx[c][:, 1:1 + S], op=aop.mult)
                nc.tensor.matmul(p2_ps[0:1, :S], ones_col, sq[:, :S],
                                 start=(c == 0), stop=(c == EB - 1))
            phpen_f = spool.tile([1, S], f32, tag="phpen", bufs=1, name="phpen_f")
            nc.sync.dma_start(phpen_f, d_phpen[s:s + 1, :])
            # pen = -p2 + phpen   (phpen is 0 valid / -1e9 masked)
            nc.vector.scalar_tensor_tensor(
                out=pen_sb[0:1, :S], in0=p2_ps[0:1, :S], scalar=-1.0,
                in1=phpen_f[0:1, :S], op0=aop.mult, op1=aop.add)
            nc.vector.memset(pen_sb[0:1, S:], -1e9)
            for c in range(EB):
                for si, (s0, sw) in enumerate(SBLK):
                    tr = ps_tile_bf("trph_ps")
                    nc.tensor.transpose(tr[:sw, :128],
                                        phx[c][:, 1 + s0:1 + s0 + sw],
                                        ident)
                    nc.scalar.copy(phT[si][:sw, c * 128:(c + 1) * 128],
                                   tr[:sw, :128])

            # ---- mel projection ----
            mels_sb = spool.tile([MEL_DIMS, T], bf16, tag="mels", name="mels_sb")
            nc.sync.dma_start(mels_sb[:, :Teff], d_mels[s, :, :Teff])
            for (t0, W) in TTs:
                for eb in range(EB):
                    ps = ps_tile("proj_ps")
                    nc.tensor.matmul(ps[:, :W],
                                     meproj_sb[:, eb * 128:(eb + 1) * 128],
                                     mels_sb[:, t0:t0 + W], start=True, stop=True)
                    nc.scalar.activation(decx[eb][:, 1 + t0:1 + t0 + W], ps[:, :W],
                                         AF.Identity, bias=b_meproj[:, eb:eb + 1],
                                         scale=1.0)

            def attn_group(gi, s=s, decx=decx):
                g0, GW = TTs[gi]
                atg = [spool.tile([128, 512], bf16, tag=f"attnT{si}",
                                  name=f"attnT{si}") for si in range(len(SBLK))]
                for t0 in range(g0, g0 + GW, 128):
                    TW = min(128, g0 + GW - t0)
                    j = (t0 - g0) // 128
                    s_ps = ps_tile("s_ps")
                    for c in range(EB):
                        nc.tensor.matmul(s_ps[:TW, :S],
                                         decx[c][:, 1 + t0:1 + t0 + TW],
                                         ph2[c][:, :S], start=(c == 0), stop=False)
                    nc.tensor.matmul(s_ps[:TW, :S], ones_row[:, :TW],
                                     pen_sb[0:1, :S], start=False, stop=True)
                    negmx = spool.tile([128, 1], f32, tag="negmx", bufs=4, name="negmx")
                    nc.vector.reduce_max(negmx[:TW], s_ps[:TW, :S], axis=AX.X,
                                         negate=True)
                    attn = spool.tile([128, S], bf16, tag="attn", bufs=4, name="attn")
                    sumexp = spool.tile([128, 1], f32, tag="sumexp", bufs=4,
                                        name="sumexp")
                    nc.scalar.activation(attn[:TW, :S], s_ps[:TW, :S], AF.Exp,
                                         bias=negmx[:TW], scale=1.0,
                                         accum_out=sumexp[:TW])
                    rcp = spool.tile([128, 1], f32, tag="rcp", bufs=4, name="rcp")
                    nc.vector.reciprocal(rcp[:TW], sumexp[:TW])
                    nc.vector.tensor_scalar_mul(attn[:TW, :S], attn[:TW, :S],
                                                rcp[:TW])
                    for si, (s0, sw) in enumerate(SBLK):
                        tr = ps_tile_bf("trat_ps")
                        nc.tensor.transpose(tr[:sw, :TW], attn[:TW, s0:s0 + sw],
                                            ident[:TW, :TW])
                        nc.scalar.copy(atg[si][:sw, j * 128:j * 128 + TW],
                                       tr[:sw, :TW])
                for eb in range(EB):
                    ctx = ps_tile("ctx_ps")
                    for si, (s0, sw) in enumerate(SBLK):
                        nc.tensor.matmul(ctx[:, :GW],
                                         phT[si][:sw, eb * 128:(eb + 1) * 128],
                                         atg[si][:sw, :GW],
                                         start=(si == 0), stop=(si == len(SBLK) - 1))
                    nc.vector.tensor_copy(decx[EB + eb][:, 1 + g0:1 + g0 + GW],
                                          ctx[:, :GW])

            # ---- mel encoder: 4 GLU layers on decx[0:3]; attention group g
            # is emitted under layer 3's dense convs right after the tile
            # that produces its mel_enc columns ----
            def mel3_hook(i):
                if i >= 1:
                    attn_group(i - 1)

            for l in range(4):
                wt = load_conv_weights(d_mew, l, EB, 2 * E, "mew")
                glu_layer(decx[:EB], EB, TTs, TP, melmask,
                          wa=lambda k, c, h, wt=wt: wt[(k, c, 0)][:, h * 128:(h + 1) * 128],
                          wg=lambda k, c, h, wt=wt: wt[(k, c, 0)][:, (EB + h) * 128:(EB + h + 1) * 128],
                          bias_a=lambda h, l=l: b_me[:, l, h:h + 1],
                          bias_g=lambda h, l=l: b_me[:, l, EB + h:EB + h + 1],
                          pre_tile_hook=mel3_hook if l == 3 else None)
            for g in range(max(0, len(TTs) - 1), len(TTs)):
                attn_group(g)

            # ---- attention: softmax over phonemes, context -> decx[3:6] ----
            # Emitted per 512-wide group as the pre-chunk hook of dec layer 0
            # so its sparse PE work interleaves with dense conv matmuls
            # (keeps the HAM clock warm) and context is consumed as produced.
            # ---- logits (feature-major) -> logitbf, then mel_h0 ----
            def lgprep(ti):
                t0, W = TTs[ti]
                for vb in range(VB):
                    lf = ps_tile("lf_ps")
                    for c in range(HB):
                        nc.tensor.matmul(lf[:, :W],
                                         pdlin_sb[:, c, vb * 128:(vb + 1) * 128],
                                         decx[c][:, 1 + t0:1 + t0 + W],
                                         start=(c == 0), stop=(c == HB - 1))
                    nc.scalar.activation(logitbf[vb][:, 1 + t0:1 + t0 + W],
                                         lf[:, :W], AF.Identity,
                                         bias=b_pdlin[:, vb:vb + 1], scale=1.0)
                mh = ps_tile("mh_ps")
                for vb in range(VB):
                    nc.tensor.matmul(mh[:, :W], mdproj_sb[:, vb, :],
                                     logitbf[vb][:, 1 + t0:1 + t0 + W],
                                     start=(vb == 0), stop=(vb == VB - 1))
                nc.scalar.activation(mdx[0][:, 1 + t0:1 + t0 + W], mh[:, :W],
                                     AF.Identity, bias=b_mdproj[:, 0:1], scale=1.0)

            # ---- log_softmax blocks (emitted interleaved with md conv) ----
            nblk = len(TBLKs)
            se2a = spool.tile([128, 32], f32, tag="se2a", name="se2a")
            nc.vector.memset(se2a[:, :nblk], 1.0)
            lna = spool.tile([128, 32], f32, tag="lna", name="lna")
            outlps = []

            def lg_block(tb, s=s, se2a=se2a, outlps=outlps, decx=decx):
                t0, TW = TBLKs[tb]
                lg = ps_tile("lg_ps")
                for c in range(HB):
                    nc.tensor.matmul(lg[:TW, :V], decx[c][:, 1 + t0:1 + t0 + TW],
                                     pdlin_sb[:, c, :], start=(c == 0), stop=False)
                nc.tensor.matmul(lg[:TW, :V], ones_row[:, :TW], pdlinb_row,
                                 start=False, stop=True)
                negmx2 = spool.tile([128, 1], f32, tag="negmx2", bufs=3, name="negmx2")
                nc.vector.reduce_max(negmx2[:TW], lg[:TW, :V], axis=AX.X,
                                     negate=True)
                esc = spool.tile([128, V], bf16, tag="esc", name="esc")
                nc.scalar.activation(esc[:TW, :V], lg[:TW, :V], AF.Exp,
                                     bias=negmx2[:TW], scale=1.0,
                                     accum_out=se2a[:TW, tb:tb + 1])
                outlp = spool.tile([128, V], f32, tag="outlp", bufs=16, name="outlp")
                nc.vector.tensor_scalar_add(outlp[:TW, :V], lg[:TW, :V], negmx2[:TW])
                outlps.append(outlp)

            def preds_block(tb, s=s, mdx=mdx):
                t0, TW = TBLKs[tb]
                mp = ps_tile("mp_ps")
                nc.tensor.matmul(mp[:TW, :MEL_DIMS], mdx[0][:, 1 + t0:1 + t0 + TW],
                                 mdlin_sb[:, :MEL_DIMS], start=True, stop=False)
                nc.tensor.matmul(mp[:TW, :MEL_DIMS], ones_row[:, :TW], mdlinb_row,
                                 start=False, stop=True)
                outmp = spool.tile([128, MEL_DIMS], f32, tag="outmp", bufs=3,
                                   name="outmp")
                nc.scalar.activation(outmp[:TW, :MEL_DIMS], mp[:TW, :MEL_DIMS],
                                     AF.Tanh)
                nc.sync.dma_start(d_out[s, t0:t0 + TW, V:V + MEL_DIMS],
                                  outmp[:TW, :MEL_DIMS])
                outmps[tb] = outmp

            outmps = {}
            blocks_of = lambda i: range(4 * i, min(4 * i + 4, nblk))

            def dec1_hook(i):
                # tile i-1 of dec output is final: emit its logits work under
                # this tile's dense conv matmuls
                if i >= 1:
                    lgprep(i - 1)
                    for tb in blocks_of(i - 1):
                        lg_block(tb)

            # ---- decoder: 2 GLU layers on decx[0:6] ----
            for l in range(2):
                wt = load_conv_weights(d_pdw, l, HB, 4 * E, "pdw")
                glu_layer(decx, HB, TTs, TP, melmask,
                          wa=lambda k, c, h, wt=wt: wt[(k, c, 0)][:, h * 128:(h + 1) * 128],
                          wg=lambda k, c, h, wt=wt: wt[(k, c, 1)][:, h * 128:(h + 1) * 128],
                          bias_a=lambda h, l=l: b_pd[:, l, h:h + 1],
                          bias_g=lambda h, l=l: b_pd[:, l, HB + h:HB + h + 1],
                          pre_tile_hook=dec1_hook if l == 1 else None)
            lgprep(len(TTs) - 1)
            for tb in blocks_of(len(TTs) - 1):
                lg_block(tb)

            blocks_of = lambda i: range(4 * i, min(4 * i + 4, nblk))

            def md_hook1(i):
                if i > 0:
                    for tb in blocks_of(i - 1):
                        preds_block(tb)

            # ---- mel decoder: 2 GLU layers on mdx, sparse phases woven in ----
            for l in range(2):
                wt = load_conv_weights(d_mdw, l, 1, 2 * DEC_H, "mdw")
                glu_layer(mdx, 1, TTs, TP, melmask,
                          wa=lambda k, c, h, wt=wt: wt[(k, c, 0)][:, h * 128:(h + 1) * 128],
                          wg=lambda k, c, h, wt=wt: wt[(k, c, 0)][:, (1 + h) * 128:(2 + h) * 128],
                          bias_a=lambda h, l=l: b_md[:, l, h:h + 1],
                          bias_g=lambda h, l=l: b_md[:, l, 1 + h:2 + h],
                          pre_tile_hook=md_hook1 if l == 1 else None)
            for tb in blocks_of(len(TTs) - 1):
                preds_block(tb)

            # ---- batched ln + log_probs output ----
            nc.scalar.activation(lna[:, :nblk], se2a[:, :nblk], AF.Ln)
            for tb, (t0, TW) in enumerate(TBLKs):
                outlp = outlps[tb]
                nc.vector.tensor_scalar_sub(outlp[:TW, :V], outlp[:TW, :V],
                                            lna[:TW, tb:tb + 1])
                nc.sync.dma_start(d_out[s, t0:t0 + TW, 0:V], outlp[:TW, :V])

            if Teff < T:
                # broadcast the constant row Teff-1 over the skipped tail:
                # replicate it across partitions, then tile DMAs of <=128 rows
                row = TBLKs[-1][1] - 1
                rowcat = spool.tile([128, V + MEL_DIMS], f32, tag="rowcat",
                                    bufs=1, name="rowcat")
                nc.sync.dma_start(rowcat[0:1, 0:V], outlps[-1][row:row + 1, :V])
                nc.sync.dma_start(rowcat[0:1, V:V + MEL_DIMS],
                                  outmps[len(TBLKs) - 1][row:row + 1, :MEL_DIMS])
                nc.gpsimd.partition_broadcast(rowcat[:, :], rowcat[0:1, :])
                for r0 in range(Teff, T, 128):
                    rw = min(128, T - r0)
                    nc.sync.dma_start(d_out[s, r0:r0 + rw, :], rowcat[:rw, :])

        psB.release()
        psA.release()
        spool.release()
        xring.release()
        apool.release()
        wring.release()
        cpool.release()

    nc.compile()
    return nc


def plan_slots(mel_lens, ns=NS, T=T_MEL, n_cores=N_CORES, margin=12):
    """Sort samples by length; slot j of every core gets rank 8j+core.
    Returns (order, teffs): order[core*ns + slot] = original sample index,
    teffs[slot] = compile-time effective length for that slot (same on all
    cores, so a single SPMD NEFF serves all 8)."""
    mel_lens = np.asarray(mel_lens).astype(np.int64)
    idx = np.argsort(-mel_lens, kind='stable')
    order = np.empty(ns * n_cores, np.int64)
    teffs = []
    for j in range(ns):
        grp = idx[j * n_cores:(j + 1) * n_cores]
        for c in range(n_cores):
            order[c * ns + j] = grp[c]
        te = int(mel_lens[grp].max()) + margin
        te = min(T, ((te + 127) // 128) * 128)
        teffs.append(te)
    return order, tuple(teffs)


def preprocess(inputs, ns=NS, T=T_MEL, TPH=T_PHON, n_cores=N_CORES, order=None):
    """Host-side prep: transpose/pad/cast, build masks, shard per core."""
    S = TPH + 1
    TP = T + 2
    SP = S + 2
    B = ns * n_cores

    mels = np.asarray(inputs['mels'], np.float32)[:B, :T]
    phonemes = np.asarray(inputs['phonemes']).astype(np.int64)[:B, :TPH]
    mel_lens = np.asarray(inputs['mel_lens']).astype(np.int64)[:B]
    phoneme_lens = np.asarray(inputs['phoneme_lens']).astype(np.int64)[:B]
    if order is not None:
        mels = mels[order]
        phonemes = phonemes[order]
        mel_lens = mel_lens[order]
        phoneme_lens = phoneme_lens[order]
    emb = np.asarray(inputs['emb'], np.float32)

    mels_t = np.ascontiguousarray(mels.transpose(0, 2, 1)).astype(BF)  # [B,80,T]

    ph = np.concatenate([np.zeros((B, 1), np.int64), phonemes], axis=1)  # [B,S]
    embph = emb[ph]                                    # [B, S, E] f32
    embph_t = np.zeros((B, E, SP), np.float32)
    embph_t[:, :, 1:1 + S] = embph.transpose(0, 2, 1)
    embph_t = embph_t.astype(BF)

    t_idx = np.arange(T)
    melmask = np.zeros((B, TP), np.float32)
    melmask[:, 1:1 + T] = (t_idx[None, :] < mel_lens[:, None]).astype(np.float32)
    melmask = melmask.astype(BF)

    s_idx = np.arange(S)
    ph_valid = s_idx[None, :] <= phoneme_lens[:, None]
    phmask = np.zeros((B, SP), np.float32)
    phmask[:, 1:1 + S] = ph_valid.astype(np.float32)
    phmask = phmask.astype(BF)
    phpen = np.where(ph_valid, 0.0, -1e9).astype(np.float32)  # [B, S]

    shared = {
        'me_proj': np.asarray(inputs['me_proj_W'], np.float32).astype(BF),
        'me_w': np.asarray(inputs['me_W'], np.float32).astype(BF),
        'pe_w': np.asarray(inputs['pe_W'], np.float32).astype(BF),
        'pd_w': np.asarray(inputs['pd_W'], np.float32).astype(BF),
        'md_w': np.asarray(inputs['md_W'], np.float32).astype(BF),
        'pd_lin': np.asarray(inputs['pd_lin_W'], np.float32).astype(BF),
        'md_proj': np.asarray(inputs['md_proj_W'], np.float32).astype(BF),
        'md_lin': np.asarray(inputs['md_lin_W'], np.float32).astype(BF),
        'pd_lin_b_row': np.asarray(inputs['pd_lin_b'], np.float32)[None, :].astype(BF),
        'md_lin_b_row': np.asarray(inputs['md_lin_b'], np.float32)[None, :].astype(BF),
        'me_proj_b': np.asarray(inputs['me_proj_b'], np.float32),
        'me_b': np.asarray(inputs['me_b'], np.float32),
        'pe_b': np.asarray(inputs['pe_b'], np.float32),
        'pd_b': np.asarray(inputs['pd_b'], np.float32),
        'md_b': np.asarray(inputs['md_b'], np.float32),
        'md_proj_b': np.asarray(inputs['md_proj_b'], np.float32),
        'pd_lin_b': np.asarray(inputs['pd_lin_b'], np.float32),
    }

    in_maps = []
    for core in range(n_cores):
        sl = slice(core * ns, (core + 1) * ns)
        m = dict(shared)
        m['mels'] = np.ascontiguousarray(mels_t[sl])
        m['embph'] = np.ascontiguousarray(embph_t[sl])
        m['melmask'] = np.ascontiguousarray(melmask[sl])
        m['phmask'] = np.ascontiguousarray(phmask[sl])
        m['phpen'] = np.ascontiguousarray(phpen[sl])
        in_maps.append(m)
    return in_maps


_CACHE = {}


def _get_nc(teffs=None):
    key = teffs if teffs is not None else ('full',)
    if key not in _CACHE:
        _CACHE[key] = build(teffs=list(teffs) if teffs is not None else None)
    return _CACHE[key]


def kernel(**inputs) -> np.ndarray:
    from concourse.bass_utils import run_bass_kernel_spmd
    order, teffs = plan_slots(np.asarray(inputs['mel_lens']))
    nc = _get_nc(teffs)
    in_maps = preprocess(inputs, order=order)
    res = run_bass_kernel_spmd(nc, in_maps, core_ids=list(range(N_CORES)))
    out = np.concatenate([r['out'] for r in res.results], axis=0)
    inv = np.empty_like(order)
    inv[order] = np.arange(len(order))
    out = out[inv]
    return np.ascontiguousarray(out.astype(np.float32))


if __name__ == '__main__':
    import reference
    inputs = reference.setup_inputs()
    inputs = {k: np.asarray(v) for k, v in inputs.items()}
    out = kernel(**inputs)
    print(out.shape, out.dtype)



# revision 14
# speedup vs baseline: 1.1663x; 1.1663x over previous
"""Bass/Tile kernel for nn_AligningModel on 8 Trainium2 NeuronCores.

Data parallel: 32 samples sharded 4-per-core across 8 cores; all params
replicated.  Inside each core the model runs feature-major ([C, T] layout)
with bf16 matmul operands and fp32 PSUM accumulation:

  mel GLU encoder (4 layers)  -> mel_enc   [384, T]
  phoneme GLU encoder (4)     -> ph_enc    [384, 257]
  -L2^2 attention softmax     -> context   [384, T]   (the |mel|^2 term
      cancels inside softmax, so scores are 2*mel.ph - |ph|^2 - penalty)
  dec GLU (2 layers, 768ch)   -> dec       [768, T]
  logits twice (token-major for log_softmax, feature-major for mel decoder)
  mel GLU decoder (2 layers, 128ch) -> tanh mel preds

Convolutions (k=3, SAME) are shifted matmuls accumulated in PSUM.
"""

import numpy as np
import ml_dtypes

BF = ml_dtypes.bfloat16
F8 = ml_dtypes.float8_e4m3   # TRN fp8_e4m3 (max normal 240)
GW_SCALE = 4.0               # gate weights scaled x4 before fp8 quantization

B_FULL = 32
T_MEL = 2000
T_PHON = 256
E = 384
V = 256
MEL_DIMS = 80
DEC_H = 128
N_CORES = 8
NS = B_FULL // N_CORES  # samples per core


def _ttiles(T, w=512):
    out, t0 = [], 0
    while t0 < T:
        out.append((t0, min(w, T - t0)))
        t0 += w
    return out


def build(ns=NS, T=T_MEL, TPH=T_PHON, teffs=None):
    """Build and compile the per-core Bass kernel (ns samples, seq len T)."""
    import concourse.bacc as bacc
    import concourse.tile as tile
    import concourse.mybir as mybir
    from concourse.alu_op_type import AluOpType as aop
    from concourse.masks import make_identity
    from concourse import bass_isa

    f32 = mybir.dt.float32
    bf16 = mybir.dt.bfloat16
    f8e4 = mybir.dt.float8e4
    DR = mybir.MatmulPerfMode.DoubleRow
    AF = mybir.ActivationFunctionType
    AX = mybir.AxisListType

    if teffs is None:
        teffs = [T] * ns
    assert len(teffs) == ns and all(128 <= t <= T and t % 128 == 0 or t == T
                                    for t in teffs)
    S = TPH + 1          # phoneme positions incl. prepended blank
    TP = T + 2           # zero border column each side
    SP = S + 2
    TT = _ttiles(T)          # 512-wide t tiles
    TBLK = _ttiles(T, 128)   # 128-wide t blocks
    SBLK = _ttiles(S, 128)   # s chunks
    EB = E // 128            # 3
    HB = 2 * E // 128        # 6
    VB = V // 128            # 2
    XMW = 516                # xm chunk slot width (512 + 2 halo + margin)

    nc = bacc.Bacc("TRN2", debug=False, target_bir_lowering=False)

    # ---------------- DRAM I/O ----------------
    d_mels = nc.dram_tensor("mels", [ns, MEL_DIMS, T], bf16, kind="ExternalInput").ap()
    d_embph = nc.dram_tensor("embph", [ns, E, SP], bf16, kind="ExternalInput").ap()
    d_melmask = nc.dram_tensor("melmask", [ns, TP], bf16, kind="ExternalInput").ap()
    d_phmask = nc.dram_tensor("phmask", [ns, SP], bf16, kind="ExternalInput").ap()
    d_phpen = nc.dram_tensor("phpen", [ns, S], f32, kind="ExternalInput").ap()

    d_meproj = nc.dram_tensor("me_proj", [MEL_DIMS, E], bf16, kind="ExternalInput").ap()
    d_mew = nc.dram_tensor("me_w", [4, 3, E, 2 * E], bf16, kind="ExternalInput").ap()
    d_pew = nc.dram_tensor("pe_w", [4, 3, E, 2 * E], bf16, kind="ExternalInput").ap()
    d_pdwa = nc.dram_tensor("pd_wa", [2, 3, 2 * E, 2 * E], bf16, kind="ExternalInput").ap()
    d_pdwg8 = nc.dram_tensor("pd_wg8", [2, 3, 2 * E, 2 * E], f8e4, kind="ExternalInput").ap()
    d_mdw = nc.dram_tensor("md_w", [2, 3, DEC_H, 2 * DEC_H], bf16, kind="ExternalInput").ap()
    d_pdlin = nc.dram_tensor("pd_lin", [2 * E, V], bf16, kind="ExternalInput").ap()
    d_mdproj = nc.dram_tensor("md_proj", [V, DEC_H], bf16, kind="ExternalInput").ap()
    d_mdlin = nc.dram_tensor("md_lin", [DEC_H, MEL_DIMS], bf16, kind="ExternalInput").ap()
    d_mdlinb_row = nc.dram_tensor("md_lin_b_row", [1, MEL_DIMS], bf16, kind="ExternalInput").ap()

    d_meprojb = nc.dram_tensor("me_proj_b", [E], f32, kind="ExternalInput").ap()
    d_meb = nc.dram_tensor("me_b", [4, 2 * E], f32, kind="ExternalInput").ap()
    d_peb = nc.dram_tensor("pe_b", [4, 2 * E], f32, kind="ExternalInput").ap()
    d_pdb = nc.dram_tensor("pd_b", [2, 4 * E], f32, kind="ExternalInput").ap()
    d_mdb = nc.dram_tensor("md_b", [2, 2 * DEC_H], f32, kind="ExternalInput").ap()
    d_mdprojb = nc.dram_tensor("md_proj_b", [DEC_H], f32, kind="ExternalInput").ap()
    d_pdlinb = nc.dram_tensor("pd_lin_b", [V], f32, kind="ExternalInput").ap()

    d_out = nc.dram_tensor("out", [ns, T, V + MEL_DIMS], f32, kind="ExternalOutput").ap()

    with tile.TileContext(nc) as tc:
        cpool = tc.alloc_tile_pool(name="consts", bufs=1)
        wring = tc.alloc_tile_pool(name="wring", bufs=30)
        apool = tc.alloc_tile_pool(name="acts", bufs=1)
        xring = tc.alloc_tile_pool(name="xmring", bufs=12)
        spool = tc.alloc_tile_pool(name="scratch", bufs=2)
        psA = tc.alloc_tile_pool(name="psumA", bufs=6, space="PSUM")
        psB = tc.alloc_tile_pool(name="psumB", bufs=2, space="PSUM")

        def ps_conv(name="ps"):
            # conv accumulators: private ring so sparse phases can't gate them
            return psA.tile([128, 512], f32, tag="psa", name=name)

        def ps_tile(name="ps"):
            return psB.tile([128, 512], f32, tag="psb", name=name)

        def ps_tile_bf(name="psb"):
            # PE transpose writes through in the input dtype
            return psB.tile([128, 512], bf16, tag="psb", name=name)

        # ---------------- constants / resident weights ----------------
        ones_row = cpool.tile([1, 512], bf16, name="ones_row")
        nc.vector.memset(ones_row, 1.0)
        ones_col = cpool.tile([128, 1], bf16, name="ones_col")
        nc.vector.memset(ones_col, 1.0)
        ident = cpool.tile([128, 128], bf16, name="ident")
        make_identity(nc, ident)

        meproj_sb = cpool.tile([MEL_DIMS, E], bf16, name="meproj_sb")
        nc.sync.dma_start(meproj_sb, d_meproj)
        pdlin_sb = cpool.tile([128, HB, V], bf16, name="pdlin_sb")
        for c in range(HB):
            nc.sync.dma_start(pdlin_sb[:, c, :], d_pdlin[c * 128:(c + 1) * 128, :])
        mdproj_sb = cpool.tile([128, VB, DEC_H], bf16, name="mdproj_sb")
        for c in range(VB):
            nc.sync.dma_start(mdproj_sb[:, c, :], d_mdproj[c * 128:(c + 1) * 128, :])
        mdlin_sb = cpool.tile([128, MEL_DIMS], bf16, name="mdlin_sb")
        nc.sync.dma_start(mdlin_sb, d_mdlin)
        mdlinb_row = cpool.tile([1, MEL_DIMS], bf16, name="mdlinb_row")
        nc.sync.dma_start(mdlinb_row, d_mdlinb_row)

        # per-partition bias tables: [128, nlayer, nblocks]
        b_meproj = cpool.tile([128, EB], f32, name="b_meproj")
        nc.sync.dma_start(b_meproj, d_meprojb.rearrange("(a p) -> p a", p=128))
        b_me = cpool.tile([128, 4, HB], f32, name="b_me")
        nc.sync.dma_start(b_me, d_meb.rearrange("l (a p) -> p l a", p=128))
        b_pe = cpool.tile([128, 4, HB], f32, name="b_pe")
        nc.sync.dma_start(b_pe, d_peb.rearrange("l (a p) -> p l a", p=128))
        b_pd = cpool.tile([128, 2, 2 * HB], f32, name="b_pd")
        nc.sync.dma_start(b_pd, d_pdb.rearrange("l (a p) -> p l a", p=128))
        b_md = cpool.tile([128, 2, 2], f32, name="b_md")
        nc.sync.dma_start(b_md, d_mdb.rearrange("l (a p) -> p l a", p=128))
        b_mdproj = cpool.tile([128, 1], f32, name="b_mdproj")
        nc.sync.dma_start(b_mdproj, d_mdprojb.rearrange("(a p) -> p a", p=128))
        b_pdlin = cpool.tile([128, VB], f32, name="b_pdlin")
        nc.sync.dma_start(b_pdlin, d_pdlinb.rearrange("(a p) -> p a", p=128))


        # ---------------- persistent activations (per-sample reuse) ------
        # decx/phx are allocated per sample below (mel half + phx double
        # buffered so sample s+1's encoders overlap sample s's decoder)
        ph2 = [apool.tile([128, 3 * 128], bf16, name=f"ph2_{c}") for c in range(EB)]
        phT = [apool.tile([128, E], bf16, name=f"phT{i}") for i in range(len(SBLK))]
        mdx = [apool.tile([128, TP], bf16, name="mdx0")]
        logitbf = [apool.tile([128, TP], bf16, name=f"logitbf{v}") for v in range(VB)]
        pen_sb = apool.tile([1, 3 * 128], bf16, name="pen_sb")

        def zero_borders(tiles):
            for t_ in tiles:
                w = t_.shape[1]
                nc.vector.memset(t_[:, 0:1], 0.0)
                nc.vector.memset(t_[:, w - 1:w], 0.0)

        zero_borders(mdx)

        # ---------------- helpers ----------------
        def load_conv_weights(dram, l, n_cin, cout_w, name):
            """Stream one conv layer's weights through the ring.

            Returns dict (k, c, j) -> tile of [128, min(768, cout_w)] where j
            indexes 768-wide chunks of the cout dim.
            """
            njc = (cout_w + 767) // 768
            cw = min(768, cout_w)
            tiles = {}
            for k in range(3):
                for c in range(n_cin):
                    for j in range(njc):
                        wt = wring.tile([128, cw], bf16, tag="wconv",
                                        name=f"{name}{l}_{k}_{c}_{j}")
                        nc.sync.dma_start(
                            wt, dram[l, k, c * 128:(c + 1) * 128,
                                     j * cw:(j + 1) * cw])
                        tiles[(k, c, j)] = wt
            return tiles

        def glu_layer(x_tiles, n_cin, tiles_list, TPAD, mask_rep, wa, wg, bias_a, bias_g,
                      pre_chunk_hook=None, pre_tile_hook=None, wg8=None):
            """One masked GLU conv block, in place on x_tiles (bf16).

            wa(k, c, h) / wg(k, c, h) -> lhsT AP [128, 128] for the a/g couts.
            bias_a(h)/bias_g(h) -> [128, 1] fp32 AP.
            Residual uses the masked input (matches reference semantics).
            pre_chunk_hook(i): emitted just before tile i's xm chunks (used to
            interleave the producer of x_tiles' columns, e.g. attention).
            pre_tile_hook(i): emitted before tile i's conv matmuls (used to
            interleave unrelated sparse-PE phases so HAM stays warm).
            wg8: optional fp8 gate path — wg8(k, p, h) -> lhsT AP [128, 2, 128]
            holding cin blocks (2p, 2p+1) of the x4-scaled gate weights; the
            gate matmuls then run in DoubleRow mode (2 k-blocks per pass) and
            the sigmoid dequantizes via scale=1/GW_SCALE.  Requires n_cin even.
            """
            n_half = n_cin  # cout == 2*cin for every GLU here
            xm = {}
            xm8 = {}

            def emit_xm(i):
                if pre_chunk_hook is not None:
                    pre_chunk_hook(i)
                t0, W = tiles_list[i]
                cw = min(W + 2, TPAD - t0)
                for c in range(n_cin):
                    xt = xring.tile([128, XMW], bf16, tag="xm", name=f"xm{c}_{i}")
                    nc.vector.tensor_tensor(
                        out=xt[:, :cw], in0=x_tiles[c][:, t0:t0 + cw],
                        in1=mask_rep[:, t0:t0 + cw], op=aop.mult)
                    xm[(c, i)] = xt
                if wg8 is not None:
                    for p in range(n_cin // 2):
                        xt8 = xring.tile([128, 2, XMW], f8e4, tag="xm8", bufs=6,
                                         name=f"xm8_{p}_{i}")
                        for m in range(2):
                            nc.vector.tensor_copy(xt8[:, m, :cw],
                                                  xm[(2 * p + m, i)][:, :cw])
                        xm8[(p, i)] = xt8

            emit_xm(0)
            for i, (t0, W) in enumerate(tiles_list):
                if i + 1 < len(tiles_list):
                    emit_xm(i + 1)
                if pre_tile_hook is not None:
                    pre_tile_hook(i)
                for h in range(n_half):
                    a_ps = ps_conv("a_ps")
                    g_ps = ps_conv("g_ps")
                    nmm = 3 * n_cin
                    idx = 0
                    for k in range(3):
                        for c in range(n_cin):
                            nc.tensor.matmul(
                                a_ps[:, :W], wa(k, c, h),
                                xm[(c, i)][:, k:k + W],
                                start=(idx == 0), stop=(idx == nmm - 1))
                            idx += 1
                    if wg8 is not None:
                        npairs = n_cin // 2
                        idx = 0
                        for k in range(3):
                            for p in range(npairs):
                                nc.tensor.matmul(
                                    g_ps[:, :W], wg8(k, p, h),
                                    xm8[(p, i)][:, :, k:k + W],
                                    start=(idx == 0), stop=(idx == 3 * npairs - 1),
                                    perf_mode=DR)
                                idx += 1
                    else:
                        idx = 0
                        for k in range(3):
                            for c in range(n_cin):
                                nc.tensor.matmul(
                                    g_ps[:, :W], wg(k, c, h),
                                    xm[(c, i)][:, k:k + W],
                                    start=(idx == 0), stop=(idx == nmm - 1))
                                idx += 1
                    sig = spool.tile([128, 512], f32, tag="sig", bufs=3, name="sig")
                    nc.scalar.activation(sig[:, :W], g_ps[:, :W], AF.Sigmoid,
                                         bias=bias_g(h),
                                         scale=(1.0 / GW_SCALE) if wg8 is not None
                                         else 1.0)
                    tmp = spool.tile([128, 512], f32, tag="tmp", bufs=3, name="tmp")
                    nc.vector.scalar_tensor_tensor(
                        out=tmp[:, :W], in0=a_ps[:, :W], scalar=bias_a(h),
                        in1=sig[:, :W], op0=aop.add, op1=aop.mult)
                    # x = tmp + xm   (masked residual, exact in bf16)
                    nc.vector.tensor_tensor(
                        out=x_tiles[h][:, 1 + t0:1 + t0 + W], in0=tmp[:, :W],
                        in1=xm[(h, i)][:, 1:1 + W], op=aop.add)

        import concourse.bass as bass

        def bcast_row(row_ap):
            # [1, N] AP -> stride-0 partition broadcast AP [128, N] for DMA
            return bass.AP(tensor=row_ap.tensor, offset=row_ap.offset,
                           ap=[[0, 128]] + [list(d) for d in row_ap.ap[1:]])

        # ================ per-sample pipeline ================
        for s in range(ns):
            # beyond mel_len the masked convs see zeros, so everything is a
            # per-channel constant: compute t < Teff only, broadcast the tail
            Teff = teffs[s]
            TTs = _ttiles(Teff)
            TBLKs = _ttiles(Teff, 128)
            decx = [apool.tile([128, TP], bf16, tag=f"decx{c}",
                               bufs=2 if c < EB else 1, name=f"decx{c}")
                    for c in range(HB)]
            phx = [apool.tile([128, SP], bf16, tag=f"phx{c}", bufs=2,
                              name=f"phx{c}") for c in range(EB)]
            zero_borders(decx + phx)
            if s < 2 and Teff < T:
                for t_ in decx + mdx + logitbf:
                    nc.vector.memset(t_[:, 1 + Teff:TP - 1], 0.0)
            # ---- masks (partition-broadcast via stride-0 DMA) ----
            melmask = spool.tile([128, TP], bf16, tag="melmask", name="melmask")
            nc.sync.dma_start(out=melmask, in_=bcast_row(d_melmask[s:s + 1, :]))
            phmask = spool.tile([128, SP], bf16, tag="phmask", name="phmask")
            nc.sync.dma_start(out=phmask, in_=bcast_row(d_phmask[s:s + 1, :]))

            # ---- phoneme embedding + encoder (first: cheap DMA, fills the
            # pipe while mel data loads) ----
            for c in range(EB):
                nc.sync.dma_start(phx[c], d_embph[s, c * 128:(c + 1) * 128, :])
            for l in range(4):
                wt = load_conv_weights(d_pew, l, EB, 2 * E, "pew")
                glu_layer(phx, EB, [(0, S)], SP, phmask,
                          wa=lambda k, c, h, wt=wt: wt[(k, c, 0)][:, h * 128:(h + 1) * 128],
                          wg=lambda k, c, h, wt=wt: wt[(k, c, 0)][:, (EB + h) * 128:(EB + h + 1) * 128],
                          bias_a=lambda h, l=l: b_pe[:, l, h:h + 1],
                          bias_g=lambda h, l=l: b_pe[:, l, EB + h:EB + h + 1])

            # ---- attention prep: ph2 = 2*ph_enc, p2 = sum(ph^2), phT ----
            p2_ps = ps_tile("p2_ps")
            for c in range(EB):
                nc.vector.tensor_scalar_mul(ph2[c][:, :S], phx[c][:, 1:1 + S], 2.0)
                nc.vector.memset(ph2[c][:, S:], 0.0)
                sq = spool.tile([128, S], bf16, tag="sq", name="sq")
                nc.vector.tensor_tensor(sq[:, :S], phx[c][:, 1:1 + S],
                                        phx[c][:, 1:1 + S], op=aop.mult)
                nc.tensor.matmul(p2_ps[0:1, :S], ones_col, sq[:, :S],
                                 start=(c == 0), stop=(c == EB - 1))
            phpen_f = spool.tile([1, S], f32, tag="phpen", bufs=1, name="phpen_f")
            nc.sync.dma_start(phpen_f, d_phpen[s:s + 1, :])
            # pen = -p2 + phpen   (phpen is 0 valid / -1e9 masked)
            nc.vector.scalar_tensor_tensor(
                out=pen_sb[0:1, :S], in0=p2_ps[0:1, :S], scalar=-1.0,
                in1=phpen_f[0:1, :S], op0=aop.mult, op1=aop.add)
            nc.vector.memset(pen_sb[0:1, S:], -1e9)
            for c in range(EB):
                for si, (s0, sw) in enumerate(SBLK):
                    tr = ps_tile_bf("trph_ps")
                    nc.tensor.transpose(tr[:sw, :128],
                                        phx[c][:, 1 + s0:1 + s0 + sw],
                                        ident)
                    nc.scalar.copy(phT[si][:sw, c * 128:(c + 1) * 128],
                                   tr[:sw, :128])

            # ---- mel projection ----
            mels_sb = spool.tile([MEL_DIMS, T], bf16, tag="mels", name="mels_sb")
            nc.sync.dma_start(mels_sb[:, :Teff], d_mels[s, :, :Teff])
            for (t0, W) in TTs:
                for eb in range(EB):
                    ps = ps_tile("proj_ps")
                    nc.tensor.matmul(ps[:, :W],
                                     meproj_sb[:, eb * 128:(eb + 1) * 128],
                                     mels_sb[:, t0:t0 + W], start=True, stop=True)
                    nc.scalar.activation(decx[eb][:, 1 + t0:1 + t0 + W], ps[:, :W],
                                         AF.Identity, bias=b_meproj[:, eb:eb + 1],
                                         scale=1.0)

            def attn_group(gi, s=s, decx=decx):
                g0, GW = TTs[gi]
                atg = [spool.tile([128, 512], bf16, tag=f"attnT{si}",
                                  name=f"attnT{si}") for si in range(len(SBLK))]
                for t0 in range(g0, g0 + GW, 128):
                    TW = min(128, g0 + GW - t0)
                    j = (t0 - g0) // 128
                    s_ps = ps_tile("s_ps")
                    for c in range(EB):
                        nc.tensor.matmul(s_ps[:TW, :S],
                                         decx[c][:, 1 + t0:1 + t0 + TW],
                                         ph2[c][:, :S], start=(c == 0), stop=False)
                    nc.tensor.matmul(s_ps[:TW, :S], ones_row[:, :TW],
                                     pen_sb[0:1, :S], start=False, stop=True)
                    negmx = spool.tile([128, 1], f32, tag="negmx", bufs=4, name="negmx")
                    nc.vector.reduce_max(negmx[:TW], s_ps[:TW, :S], axis=AX.X,
                                         negate=True)
                    attn = spool.tile([128, S], bf16, tag="attn", bufs=4, name="attn")
                    sumexp = spool.tile([128, 1], f32, tag="sumexp", bufs=4,
                                        name="sumexp")
                    nc.scalar.activation(attn[:TW, :S], s_ps[:TW, :S], AF.Exp,
                                         bias=negmx[:TW], scale=1.0,
                                         accum_out=sumexp[:TW])
                    rcp = spool.tile([128, 1], f32, tag="rcp", bufs=4, name="rcp")
                    nc.vector.reciprocal(rcp[:TW], sumexp[:TW])
                    nc.vector.tensor_scalar_mul(attn[:TW, :S], attn[:TW, :S],
                                                rcp[:TW])
                    for si, (s0, sw) in enumerate(SBLK):
                        tr = ps_tile_bf("trat_ps")
                        nc.tensor.transpose(tr[:sw, :TW], attn[:TW, s0:s0 + sw],
                                            ident[:TW, :TW])
                        nc.scalar.copy(atg[si][:sw, j * 128:j * 128 + TW],
                                       tr[:sw, :TW])
                for eb in range(EB):
                    ctx = ps_tile("ctx_ps")
                    for si, (s0, sw) in enumerate(SBLK):
                        nc.tensor.matmul(ctx[:, :GW],
                                         phT[si][:sw, eb * 128:(eb + 1) * 128],
                                         atg[si][:sw, :GW],
                                         start=(si == 0), stop=(si == len(SBLK) - 1))
                    nc.vector.tensor_copy(decx[EB + eb][:, 1 + g0:1 + g0 + GW],
                                          ctx[:, :GW])

            # ---- mel encoder: 4 GLU layers on decx[0:3]; attention group g
            # is emitted under layer 3's dense convs right after the tile
            # that produces its mel_enc columns ----
            def mel3_hook(i):
                if i >= 1:
                    attn_group(i - 1)

            for l in range(4):
                wt = load_conv_weights(d_mew, l, EB, 2 * E, "mew")
                glu_layer(decx[:EB], EB, TTs, TP, melmask,
                          wa=lambda k, c, h, wt=wt: wt[(k, c, 0)][:, h * 128:(h + 1) * 128],
                          wg=lambda k, c, h, wt=wt: wt[(k, c, 0)][:, (EB + h) * 128:(EB + h + 1) * 128],
                          bias_a=lambda h, l=l: b_me[:, l, h:h + 1],
                          bias_g=lambda h, l=l: b_me[:, l, EB + h:EB + h + 1],
                          pre_tile_hook=mel3_hook if l == 3 else None)
            for g in range(max(0, len(TTs) - 1), len(TTs)):
                attn_group(g)

            # ---- attention: softmax over phonemes, context -> decx[3:6] ----
            # Emitted per 512-wide group as the pre-chunk hook of dec layer 0
            # so its sparse PE work interleaves with dense conv matmuls
            # (keeps the HAM clock warm) and context is consumed as produced.
            # ---- logits (feature-major) -> logitbf, then mel_h0 ----
            def lgprep(ti):
                t0, W = TTs[ti]
                for vb in range(VB):
                    lf = ps_tile("lf_ps")
                    for c in range(HB):
                        nc.tensor.matmul(lf[:, :W],
                                         pdlin_sb[:, c, vb * 128:(vb + 1) * 128],
                                         decx[c][:, 1 + t0:1 + t0 + W],
                                         start=(c == 0), stop=(c == HB - 1))
                    nc.vector.tensor_scalar_add(logitbf[vb][:, 1 + t0:1 + t0 + W],
                                                lf[:, :W], b_pdlin[:, vb:vb + 1])
                mh = ps_tile("mh_ps")
                for vb in range(VB):
                    nc.tensor.matmul(mh[:, :W], mdproj_sb[:, vb, :],
                                     logitbf[vb][:, 1 + t0:1 + t0 + W],
                                     start=(vb == 0), stop=(vb == VB - 1))
                nc.vector.tensor_scalar_add(mdx[0][:, 1 + t0:1 + t0 + W],
                                            mh[:, :W], b_mdproj[:, 0:1])

            # ---- log_softmax blocks (emitted interleaved with md conv) ----
            nblk = len(TBLKs)
            se2a = spool.tile([128, 32], f32, tag="se2a", name="se2a")
            nc.vector.memset(se2a[:, :nblk], 1.0)
            lna = spool.tile([128, 32], f32, tag="lna", name="lna")
            outlps = []

            def lg_block(tb, s=s, se2a=se2a, outlps=outlps):
                # token-major logits via PE transpose of logitbf (already
                # biased), instead of recomputing the decx @ pd_lin matmul
                t0, TW = TBLKs[tb]
                lg = ps_tile_bf("lg_ps")
                for vb in range(VB):
                    nc.tensor.transpose(lg[:TW, vb * 128:(vb + 1) * 128],
                                        logitbf[vb][:, 1 + t0:1 + t0 + TW],
                                        ident)
                negmx2 = spool.tile([128, 1], f32, tag="negmx2", bufs=3, name="negmx2")
                nc.vector.reduce_max(negmx2[:TW], lg[:TW, :V], axis=AX.X,
                                     negate=True)
                esc = spool.tile([128, V], bf16, tag="esc", name="esc")
                nc.scalar.activation(esc[:TW, :V], lg[:TW, :V], AF.Exp,
                                     bias=negmx2[:TW], scale=1.0,
                                     accum_out=se2a[:TW, tb:tb + 1])
                outlp = spool.tile([128, V], f32, tag="outlp", bufs=16, name="outlp")
                nc.vector.tensor_scalar_add(outlp[:TW, :V], lg[:TW, :V], negmx2[:TW])
                outlps.append(outlp)

            def preds_block(tb, s=s, mdx=mdx):
                t0, TW = TBLKs[tb]
                mp = ps_tile("mp_ps")
                nc.tensor.matmul(mp[:TW, :MEL_DIMS], mdx[0][:, 1 + t0:1 + t0 + TW],
                                 mdlin_sb[:, :MEL_DIMS], start=True, stop=False)
                nc.tensor.matmul(mp[:TW, :MEL_DIMS], ones_row[:, :TW], mdlinb_row,
                                 start=False, stop=True)
                outmp = spool.tile([128, MEL_DIMS], f32, tag="outmp", bufs=3,
                                   name="outmp")
                nc.scalar.activation(outmp[:TW, :MEL_DIMS], mp[:TW, :MEL_DIMS],
                                     AF.Tanh)
                nc.sync.dma_start(d_out[s, t0:t0 + TW, V:V + MEL_DIMS],
                                  outmp[:TW, :MEL_DIMS])
                outmps[tb] = outmp

            outmps = {}
            blocks_of = lambda i: range(4 * i, min(4 * i + 4, nblk))

            def dec1_hook(i):
                # tile i-1 of dec output is final: emit its logits work under
                # this tile's dense conv matmuls
                if i >= 1:
                    lgprep(i - 1)
                    for tb in blocks_of(i - 1):
                        lg_block(tb)

            # ---- decoder: 2 GLU layers on decx[0:6]; gate half in fp8
            # DoubleRow (x4-scaled weights, dequant via sigmoid scale) ----
            for l in range(2):
                wt = load_conv_weights(d_pdwa, l, HB, 2 * E, "pdwa")
                wt8 = {}
                for k in range(3):
                    for p in range(HB // 2):
                        w8 = wring.tile([128, 2, 2 * E], f8e4, tag="wconv8",
                                        bufs=12, name=f"pdwg8_{l}_{k}_{p}")
                        nc.sync.dma_start(
                            w8, d_pdwg8[l, k, p * 256:(p + 1) * 256, :]
                            .rearrange("(m p) f -> p m f", m=2))
                        wt8[(k, p)] = w8
                glu_layer(decx, HB, TTs, TP, melmask,
                          wa=lambda k, c, h, wt=wt: wt[(k, c, 0)][:, h * 128:(h + 1) * 128],
                          wg=None,
                          bias_a=lambda h, l=l: b_pd[:, l, h:h + 1],
                          bias_g=lambda h, l=l: b_pd[:, l, HB + h:HB + h + 1],
                          pre_tile_hook=dec1_hook if l == 1 else None,
                          wg8=lambda k, p, h, wt8=wt8:
                          wt8[(k, p)][:, :, h * 128:(h + 1) * 128])
            lgprep(len(TTs) - 1)
            for tb in blocks_of(len(TTs) - 1):
                lg_block(tb)

            blocks_of = lambda i: range(4 * i, min(4 * i + 4, nblk))

            def md_hook1(i):
                if i > 0:
                    for tb in blocks_of(i - 1):
                        preds_block(tb)

            # ---- mel decoder: 2 GLU layers on mdx, sparse phases woven in ----
            for l in range(2):
                wt = load_conv_weights(d_mdw, l, 1, 2 * DEC_H, "mdw")
                glu_layer(mdx, 1, TTs, TP, melmask,
                          wa=lambda k, c, h, wt=wt: wt[(k, c, 0)][:, h * 128:(h + 1) * 128],
                          wg=lambda k, c, h, wt=wt: wt[(k, c, 0)][:, (1 + h) * 128:(2 + h) * 128],
                          bias_a=lambda h, l=l: b_md[:, l, h:h + 1],
                          bias_g=lambda h, l=l: b_md[:, l, 1 + h:2 + h],
                          pre_tile_hook=md_hook1 if l == 1 else None)
            for tb in blocks_of(len(TTs) - 1):
                preds_block(tb)

            # ---- batched ln + log_probs output ----
            nc.scalar.activation(lna[:, :nblk], se2a[:, :nblk], AF.Ln)
            for tb, (t0, TW) in enumerate(TBLKs):
                outlp = outlps[tb]
                nc.vector.tensor_scalar_sub(outlp[:TW, :V], outlp[:TW, :V],
                                            lna[:TW, tb:tb + 1])
                nc.sync.dma_start(d_out[s, t0:t0 + TW, 0:V], outlp[:TW, :V])

            if Teff < T:
                # broadcast the constant row Teff-1 over the skipped tail:
                # replicate it across partitions, then tile DMAs of <=128 rows
                row = TBLKs[-1][1] - 1
                rowcat = spool.tile([128, V + MEL_DIMS], f32, tag="rowcat",
                                    bufs=1, name="rowcat")
                nc.sync.dma_start(rowcat[0:1, 0:V], outlps[-1][row:row + 1, :V])
                nc.sync.dma_start(rowcat[0:1, V:V + MEL_DIMS],
                                  outmps[len(TBLKs) - 1][row:row + 1, :MEL_DIMS])
                nc.gpsimd.partition_broadcast(rowcat[:, :], rowcat[0:1, :])
                for r0 in range(Teff, T, 128):
                    rw = min(128, T - r0)
                    nc.sync.dma_start(d_out[s, r0:r0 + rw, :], rowcat[:rw, :])

        psB.release()
        psA.release()
        spool.release()
        xring.release()
        apool.release()
        wring.release()
        cpool.release()

    nc.compile()
    return nc


def plan_slots(mel_lens, ns=NS, T=T_MEL, n_cores=N_CORES, margin=12):
    """Sort samples by length; slot j of every core gets rank 8j+core.
    Returns (order, teffs): order[core*ns + slot] = original sample index,
    teffs[slot] = compile-time effective length for that slot (same on all
    cores, so a single SPMD NEFF serves all 8)."""
    mel_lens = np.asarray(mel_lens).astype(np.int64)
    idx = np.argsort(-mel_lens, kind='stable')
    order = np.empty(ns * n_cores, np.int64)
    teffs = []
    for j in range(ns):
        grp = idx[j * n_cores:(j + 1) * n_cores]
        for c in range(n_cores):
            order[c * ns + j] = grp[c]
        te = int(mel_lens[grp].max()) + margin
        te = min(T, ((te + 127) // 128) * 128)
        teffs.append(te)
    return order, tuple(teffs)


def preprocess(inputs, ns=NS, T=T_MEL, TPH=T_PHON, n_cores=N_CORES, order=None):
    """Host-side prep: transpose/pad/cast, build masks, shard per core."""
    S = TPH + 1
    TP = T + 2
    SP = S + 2
    B = ns * n_cores

    mels = np.asarray(inputs['mels'], np.float32)[:B, :T]
    phonemes = np.asarray(inputs['phonemes']).astype(np.int64)[:B, :TPH]
    mel_lens = np.asarray(inputs['mel_lens']).astype(np.int64)[:B]
    phoneme_lens = np.asarray(inputs['phoneme_lens']).astype(np.int64)[:B]
    if order is not None:
        mels = mels[order]
        phonemes = phonemes[order]
        mel_lens = mel_lens[order]
        phoneme_lens = phoneme_lens[order]
    emb = np.asarray(inputs['emb'], np.float32)

    mels_t = np.ascontiguousarray(mels.transpose(0, 2, 1)).astype(BF)  # [B,80,T]

    ph = np.concatenate([np.zeros((B, 1), np.int64), phonemes], axis=1)  # [B,S]
    embph = emb[ph]                                    # [B, S, E] f32
    embph_t = np.zeros((B, E, SP), np.float32)
    embph_t[:, :, 1:1 + S] = embph.transpose(0, 2, 1)
    embph_t = embph_t.astype(BF)

    t_idx = np.arange(T)
    melmask = np.zeros((B, TP), np.float32)
    melmask[:, 1:1 + T] = (t_idx[None, :] < mel_lens[:, None]).astype(np.float32)
    melmask = melmask.astype(BF)

    s_idx = np.arange(S)
    ph_valid = s_idx[None, :] <= phoneme_lens[:, None]
    phmask = np.zeros((B, SP), np.float32)
    phmask[:, 1:1 + S] = ph_valid.astype(np.float32)
    phmask = phmask.astype(BF)
    phpen = np.where(ph_valid, 0.0, -1e9).astype(np.float32)  # [B, S]

    pd_w = np.asarray(inputs['pd_W'], np.float32)
    shared = {
        'me_proj': np.asarray(inputs['me_proj_W'], np.float32).astype(BF),
        'me_w': np.asarray(inputs['me_W'], np.float32).astype(BF),
        'pe_w': np.asarray(inputs['pe_W'], np.float32).astype(BF),
        'pd_wa': np.ascontiguousarray(pd_w[:, :, :, :2 * E]).astype(BF),
        'pd_wg8': np.clip(pd_w[:, :, :, 2 * E:] * GW_SCALE, -240, 240).astype(F8),
        'md_w': np.asarray(inputs['md_W'], np.float32).astype(BF),
        'pd_lin': np.asarray(inputs['pd_lin_W'], np.float32).astype(BF),
        'md_proj': np.asarray(inputs['md_proj_W'], np.float32).astype(BF),
        'md_lin': np.asarray(inputs['md_lin_W'], np.float32).astype(BF),
        'md_lin_b_row': np.asarray(inputs['md_lin_b'], np.float32)[None, :].astype(BF),
        'me_proj_b': np.asarray(inputs['me_proj_b'], np.float32),
        'me_b': np.asarray(inputs['me_b'], np.float32),
        'pe_b': np.asarray(inputs['pe_b'], np.float32),
        'pd_b': np.asarray(inputs['pd_b'], np.float32),
        'md_b': np.asarray(inputs['md_b'], np.float32),
        'md_proj_b': np.asarray(inputs['md_proj_b'], np.float32),
        'pd_lin_b': np.asarray(inputs['pd_lin_b'], np.float32),
    }

    in_maps = []
    for core in range(n_cores):
        sl = slice(core * ns, (core + 1) * ns)
        m = dict(shared)
        m['mels'] = np.ascontiguousarray(mels_t[sl])
        m['embph'] = np.ascontiguousarray(embph_t[sl])
        m['melmask'] = np.ascontiguousarray(melmask[sl])
        m['phmask'] = np.ascontiguousarray(phmask[sl])
        m['phpen'] = np.ascontiguousarray(phpen[sl])
        in_maps.append(m)
    return in_maps


_CACHE = {}


def _get_nc(teffs=None):
    key = teffs if teffs is not None else ('full',)
    if key not in _CACHE:
        _CACHE[key] = build(teffs=list(teffs) if teffs is not None else None)
    return _CACHE[key]


def kernel(**inputs) -> np.ndarray:
    from concourse.bass_utils import run_bass_kernel_spmd
    order, teffs = plan_slots(np.asarray(inputs['mel_lens']))
    nc = _get_nc(teffs)
    in_maps = preprocess(inputs, order=order)
    res = run_bass_kernel_spmd(nc, in_maps, core_ids=list(range(N_CORES)))
    out = np.concatenate([r['out'] for r in res.results], axis=0)
    inv = np.empty_like(order)
    inv[order] = np.arange(len(order))
    out = out[inv]
    return np.ascontiguousarray(out.astype(np.float32))


if __name__ == '__main__':
    import reference
    inputs = reference.setup_inputs()
    inputs = {k: np.asarray(v) for k, v in inputs.items()}
    out = kernel(**inputs)
    print(out.shape, out.dtype)

